# revision 22
# baseline (speedup 1.0000x reference)
"""Trainium2 Bass kernel for nn_AM2P_55113020342736 (retrieval_knn).

Math: the reference collapses to a single combined prototype vector v[C]:
  s_fg[b,h,w] = (q[b,:,h,w] . v) / max(||q[b,:,h,w]||, 1e-12)
  logits = stack(-s_fg/T, s_fg/T)
where
  v = BETA/T * Ghat + (1-BETA)/T * sum_m w_m * Phat_m
  Ghat   = G / max(||G||, 1e-12*(fg+EPS)),          G[c] = sum_{s,hw} sf*mask
  Phat_m = Fm / max(||Fm||, 1e-12*(msum_m+EPS)),    Fm[c] = windowed masked sum
(the msum/fg denominators cancel under l2 normalization).

Sharding:
- Support statistics: each core owns one (sample, h-half) slice with ALL 384
  channels, host-transposed to [hw=4608, C]. The prototype matrix
  F[65, 384] = W^T @ feats is computed as 36 TensorE matmuls accumulating in
  one PSUM bank, where W[hw, m] = mask AND window indicator (host-built 0/1
  f32 from the int32 mask/anchor inputs). One 100KB AllReduce then gives
  every core the complete F; each core redundantly derives the tiny v.
- Query path: data-parallel, 1 query image per core, kept f32-resident in
  SBUF; per-pixel dot & squared-norm contract over C via M=1 matmuls.

All float math runs on device; the host only slices/transposes inputs and
builds integer-derived 0/1 weight matrices and two 65-float coefficient
vectors.
"""

import numpy as np

S, C, H, W = 4, 384, 96, 96
B, M = 8, 64
HW = H * W
NCORES = 8
NP = M + 1                # 64 local prototypes + 1 global
HH = H // 2               # 48 rows per h-half
FH = HH * W               # 4608 support rows per core slice
RADII = (4, 8, 16)
BETA, TEMP, EPS = 0.3, 0.07, 1e-6
NK = 3                    # query c chunks of 128
JW = 512                  # matmul free width
QP = 4608                 # query DMA piece (9 x JW)
NPIECE = HW // QP         # 2
SQW = 1536                # square-op width
NSUP = FH // (6 * 128)    # 6 support DMA chunks of [128, 6, C+NP]
CP = C + NP               # 449 combined feats+W row


def _build_program():
    import concourse.bass as bass
    import concourse.bacc as bacc
    import concourse.mybir as mybir
    import concourse.tile as tile

    f32 = mybir.dt.float32
    bf16 = mybir.dt.bfloat16
    add = mybir.AluOpType.add
    mult = mybir.AluOpType.mult
    amax = mybir.AluOpType.max

    nc = bacc.Bacc()
    qf = nc.declare_dram_parameter("qf", [C, HW], f32, isOutput=False)
    sup = nc.declare_dram_parameter("sup", [NSUP, 128, 6, CP], f32, isOutput=False)
    tiny = nc.declare_dram_parameter("tiny", [NP, 1], f32, isOutput=False)
    wcoef = nc.declare_dram_parameter("wcoef", [NP, 1], f32, isOutput=False)
    out = nc.declare_dram_parameter("out", [2, HW // JW, JW], f32, isOutput=True)

    groups = [list(range(NCORES))]
    NJ = HW // JW  # 18

    with tile.TileContext(nc) as tc:
        with (
            tc.tile_pool(name="dram", bufs=1, space="DRAM") as dram,
            tc.tile_pool(name="constp", bufs=1) as constp,
            tc.tile_pool(name="qres", bufs=1) as qres,
            tc.tile_pool(name="work", bufs=3) as work,
            tc.tile_pool(name="psum", bufs=2, space=bass.MemorySpace.PSUM) as psum,
            tc.tile_pool(name="psum1", bufs=1, space=bass.MemorySpace.PSUM) as psum1,
        ):
            # ---- constants ----
            tiny_sb = constp.tile([NP, 1], f32)
            nc.sync.dma_start(out=tiny_sb[:], in_=tiny[:])
            wcoef_sb = constp.tile([NP, 1], f32)
            nc.sync.dma_start(out=wcoef_sb[:], in_=wcoef[:])
            ones128 = constp.tile([128, 1], bf16)
            nc.vector.memset(ones128[:], 1.0)

            # ---- support phase: F_partial[NP, C] = W^T @ feats ----
            # sup rows are [feats(C) | W(NP)] so one DMA feeds both operands
            fps = psum1.tile([NP, C], f32, tag="fps")
            for d in range(NSUP):
                ft = work.tile([128, 6, CP], f32, tag="ft", bufs=2)
                nc.sync.dma_start(out=ft[:], in_=sup[d])
                for j in range(6):
                    nc.tensor.matmul(
                        fps[:], ft[:, j, C:], ft[:, j, :C],
                        start=(d == 0 and j == 0),
                        stop=(d == NSUP - 1 and j == 5),
                    )
            fpart = constp.tile([NP, C], f32)
            nc.scalar.copy(fpart[:], fps[:])
            ar_in = dram.tile([NP, C], f32)
            ar_out = dram.tile([NP, C], f32, addr_space="Shared")
            nc.sync.dma_start(out=ar_in[:], in_=fpart[:])
            nc.gpsimd.collective_compute(
                "AllReduce", add, replica_groups=groups,
                ins=[ar_in.opt()], outs=[ar_out.opt()],
            )
            F = constp.tile([NP, C], f32)
            nc.sync.dma_start(out=F[:], in_=ar_out[:])

            # ---- coef_m = wcoef_m / max(||F_m||, tiny_m);  v = coef^T @ F ----
            F2 = constp.tile([NP, C], f32)
            n2 = constp.tile([NP, 1], f32)
            nc.scalar.activation(F2[:], F[:], mybir.ActivationFunctionType.Square,
                                 accum_out=n2[:])
            nrm = constp.tile([NP, 1], f32)
            nc.scalar.sqrt(nrm[:], n2[:])
            nc.vector.tensor_tensor(out=nrm[:], in0=nrm[:], in1=tiny_sb[:], op=amax)
            rcp = constp.tile([NP, 1], f32)
            nc.vector.reciprocal(rcp[:], nrm[:])
            coef = constp.tile([NP, 1], f32)
            nc.vector.tensor_tensor(out=coef[:], in0=rcp[:], in1=wcoef_sb[:], op=mult)
            vps = psum1.tile([1, C], f32, tag="vps")
            nc.tensor.matmul(vps[:], coef[:], F[:], start=True, stop=True)
            vrow = constp.tile([1, C], f32)
            nc.scalar.copy(vrow[:], vps[:])
            # reshape v -> [128, NK] column-per-c-chunk via a DRAM bounce
            vd = dram.tile([NK, 128], f32)
            nc.sync.dma_start(out=vd[:], in_=vrow[:])
            vcol = constp.tile([128, NK], f32)
            for k in range(NK):
                nc.sync.dma_start(out=vcol[:, k : k + 1], in_=vd[k].unsqueeze(1))

            # ---- query phase ----
            qb = [qres.tile([128, HW], f32, name=f"qb{k}") for k in range(NK)]
            norm2 = constp.tile([NJ, JW], f32)
            dots = constp.tile([NJ, JW], f32)
            # stream q pieces; squares + norm2 matmuls (v-independent)
            for p in range(NPIECE):
                q2t = []
                for k in range(NK):
                    qslice = qb[k][:, p * QP : (p + 1) * QP]
                    nc.sync.dma_start(
                        out=qslice, in_=qf[k * 128 : (k + 1) * 128, p * QP : (p + 1) * QP]
                    )
                    q2 = work.tile([128, QP], bf16, tag="q2", bufs=3)
                    for u in range(QP // SQW):
                        nc.vector.tensor_tensor(
                            out=q2[:, u * SQW : (u + 1) * SQW],
                            in0=qslice[:, u * SQW : (u + 1) * SQW],
                            in1=qslice[:, u * SQW : (u + 1) * SQW], op=mult)
                    q2t.append(q2)
                for jj in range(QP // JW):
                    j = p * (QP // JW) + jj
                    n2p = psum.tile([1, JW], f32, tag="n2p")
                    for k in range(NK):
                        nc.tensor.matmul(
                            n2p[:], ones128[:], q2t[k][:, jj * JW : (jj + 1) * JW],
                            start=(k == 0), stop=(k == NK - 1),
                        )
                    tmpn = work.tile([1, JW], f32, tag="tmpn")
                    nc.scalar.copy(tmpn[:], n2p[:])
                    nc.scalar.dma_start(out=norm2[j : j + 1, :], in_=tmpn[:])
            # dots (gated on v via vcol dependency)
            for j in range(NJ):
                dtp = psum.tile([1, JW], f32, tag="dtp")
                for k in range(NK):
                    nc.tensor.matmul(
                        dtp[:], vcol[:, k : k + 1],
                        qb[k][:, j * JW : (j + 1) * JW],
                        start=(k == 0), stop=(k == NK - 1),
                    )
                tmpd = work.tile([1, JW], f32, tag="tmpd")
                nc.scalar.copy(tmpd[:], dtp[:])
                nc.scalar.dma_start(out=dots[j : j + 1, :], in_=tmpd[:])

            # ---- epilogue: s1 = dots / max(sqrt(norm2), 1e-12); s0 = -s1 ----
            den = constp.tile([NJ, JW], f32)
            nc.scalar.sqrt(den[:], norm2[:])
            nc.vector.tensor_scalar_max(den[:], den[:], 1e-12)
            rden = constp.tile([NJ, JW], f32)
            nc.vector.reciprocal(rden[:], den[:])
            s1 = constp.tile([NJ, JW], f32)
            nc.vector.tensor_tensor(out=s1[:], in0=dots[:], in1=rden[:], op=mult)
            s0 = constp.tile([NJ, JW], f32)
            nc.scalar.mul(s0[:], s1[:], -1.0)
            nc.sync.dma_start(out=out[1], in_=s1[:])
            nc.sync.dma_start(out=out[0], in_=s0[:])

    nc.finalize()
    return nc


def prepare(support_feats, support_masks, query_feats, anchor_pos,
            anchor_sample, anchor_radius):
    """Host prep: returns (nc, in_maps)."""
    mask = support_masks[:, 0].astype(np.float32)          # [S,H,W]
    fg = float(np.float32(mask.sum()))

    # integral image of mask for windowed fg counts (host, int bookkeeping)
    ii = np.zeros((S, H + 1, W + 1), np.float64)
    ii[:, 1:, 1:] = mask.astype(np.float64).cumsum(1).cumsum(2)

    windows, msums = [], []
    for m in range(M):
        y, x = int(anchor_pos[m, 0]), int(anchor_pos[m, 1])
        s = int(anchor_sample[m])
        r = RADII[int(anchor_radius[m])]
        y1, y2 = max(y - r, 0), min(y + r, H - 1)
        x1, x2 = max(x - r, 0), min(x + r, W - 1)
        windows.append((s, y1, y2, x1, x2))
        msums.append(ii[s, y2 + 1, x2 + 1] - ii[s, y1, x2 + 1]
                     - ii[s, y2 + 1, x1] + ii[s, y1, x1])
    msums = np.asarray(msums, np.float32)

    # reference's double weight normalization, in f32 like the reference
    lw = msums / (np.float32(msums.sum()) + np.float32(EPS))
    w = lw / (np.float32(lw.sum()) + np.float32(EPS))

    tiny = np.empty((NP, 1), np.float32)
    tiny[:M, 0] = 1e-12 * (msums + np.float32(EPS))
    tiny[M, 0] = 1e-12 * (fg + EPS)
    wcoef = np.empty((NP, 1), np.float32)
    wcoef[:M, 0] = (1.0 - BETA) * w / TEMP
    wcoef[M, 0] = BETA / TEMP

    nc = _build_program()

    qfv = query_feats.reshape(B, C, HW)
    in_maps = []
    for i in range(NCORES):
        s, h = i // 2, i % 2
        # feats slice [C, HH, W] -> transposed [FH, C]
        fsl = support_feats[s, :, h * HH : (h + 1) * HH, :].reshape(C, FH)
        # W[hw, m] = mask AND (hw in window of anchor m with s_m == s);
        # col 64 = mask (global proto)
        msl = mask[s, h * HH : (h + 1) * HH, :]               # [HH, W]
        wm = np.zeros((HH, W, NP), np.float32)
        wm[:, :, M] = msl
        for m, (sm, y1, y2, x1, x2) in enumerate(windows):
            if sm != s:
                continue
            yl = max(y1 - h * HH, 0)
            yh = min(y2 - h * HH, HH - 1)
            if yl > yh:
                continue
            wm[yl : yh + 1, x1 : x2 + 1, m] = msl[yl : yh + 1, x1 : x2 + 1]
        supc = np.concatenate([fsl.T, wm.reshape(FH, NP)], axis=1)  # [FH, CP]
        in_maps.append({
            "qf": np.ascontiguousarray(qfv[i]),
            "sup": np.ascontiguousarray(supc).reshape(NSUP, 128, 6, CP),
            "tiny": tiny,
            "wcoef": wcoef,
        })
    return nc, in_maps


def assemble(results):
    outs = [np.asarray(results[i]["out"], np.float32).reshape(2, H, W)
            for i in range(NCORES)]
    return np.stack(outs, axis=0)


def kernel(support_feats, support_masks, query_feats, anchor_pos,
           anchor_sample, anchor_radius):
    from concourse.bass_utils import run_bass_kernel_spmd

    nc, in_maps = prepare(support_feats, support_masks, query_feats,
                          anchor_pos, anchor_sample, anchor_radius)
    res = run_bass_kernel_spmd(nc, in_maps, core_ids=list(range(NCORES)))
    return assemble(res.results)


if __name__ == "__main__":
    pass
